# revision 3
# baseline (speedup 1.0000x reference)
"""Trainium2 Bass kernel for nn_Attn_time (sparse time-similarity attention).

reference:
    energies[i, j] = time_sim_mat[cur[i], his[j]]   # [4096, 8192]
    out = softmax(energies, axis=-1)

Key structure exploited: cur/his index into only T=1024 time buckets, so
    out[i, j] = S[cur[i], j]  where  S = softmax_rows(time_sim_mat[:, his])
S is only [1024, 8192]. We shard S's columns (j) across the 8 cores:
each core computes S[:, jshard] ([1024, 1024]) via a one-hot matmul on the
TensorEngine (bf16), takes exp on ScalarE, AllReduces the [1024] row-sum
partials (the only cross-core traffic, 4 KiB), rescales, parks S in DRAM
(bf16) and row-gathers it by `cur` with dma_gather, casting bf16->f32
during the store of the [4096, 1024] output shard.
"""

import numpy as np

import concourse.bass as bass
import concourse.tile as tile
from concourse import bacc, mybir
from concourse.bass_utils import run_bass_kernel_spmd

T = 1024          # time buckets
SEQ = 8192        # len(his)
STATE = 4096      # len(cur)
NCORES = 8
JSH = SEQ // NCORES       # his/j columns per core = 1024
GCHUNK = 512              # gather rows per dma_gather chunk
NCHUNKS = STATE // GCHUNK  # 8

F32 = mybir.dt.float32
BF16 = mybir.dt.bfloat16
I16 = mybir.dt.int16


def build_kernel():
    nc = bacc.Bacc("TRN2", target_bir_lowering=False, debug=False,
                   num_devices=NCORES)

    m_param = nc.dram_tensor("time_sim_mat", [T, T], F32, kind="ExternalInput")
    his_param = nc.dram_tensor("his_f32", [JSH], F32, kind="ExternalInput")
    cur_param = nc.dram_tensor("cur_idx16", [128, STATE // 16], I16,
                               kind="ExternalInput")
    ucol_param = nc.dram_tensor("ucol", [128, 8], F32, kind="ExternalInput")
    out_param = nc.dram_tensor("out", [STATE, JSH], F32, kind="ExternalOutput")

    with tile.TileContext(nc, num_cores=NCORES) as tc:
        with (
            tc.tile_pool(name="singles", bufs=1) as singles,
            tc.tile_pool(name="mload", bufs=2) as mload,
            tc.tile_pool(name="gat", bufs=3) as gat,
            tc.tile_pool(name="psum", bufs=2, space="PSUM") as psum,
            tc.tile_pool(name="psrs", bufs=1, space="PSUM") as psrs,
            tc.tile_pool(name="dram", bufs=1, space="DRAM") as dram,
        ):
            # ---- persistent SBUF tiles
            mt_sb = singles.tile([128, 8, T], BF16)      # M^T: [u%128, u//128, t]
            h_sb = singles.tile([128, 8, JSH], BF16)     # one-hot his shard
            eg_sb = singles.tile([128, 8, JSH], BF16)    # exp(G), t = m*128+p
            his_sb = singles.tile([128, JSH], F32)       # his bcast to all parts
            idx_sb = singles.tile([128, STATE // 16], I16)
            ucol_sb = singles.tile([128, 8], F32)        # ucol[p,c] = c*128+p
            rs_sb = singles.tile([128, 8], F32)          # rowsum partials
            inv_sb = singles.tile([128, 8], F32)         # 1/total rowsum

            mbf_dram = dram.tile([T, T], BF16)
            s_dram = dram.tile([T, JSH], BF16)
            cc_in = dram.tile([128, 8], F32)
            cc_out = dram.tile([128, 8], F32)

            # ---- small input loads
            nc.sync.dma_start(out=idx_sb, in_=cur_param.ap())
            nc.sync.dma_start(out=ucol_sb, in_=ucol_param.ap())
            nc.sync.dma_start(
                out=his_sb,
                in_=bass.AP(tensor=his_param, offset=0, ap=[[0, 128], [1, JSH]]),
            )

            # ---- M f32 -> bf16 in DRAM (chunked through SBUF)
            for b in range(8):
                mf = mload.tile([128, T], F32)
                mb = mload.tile([128, T], BF16)
                nc.sync.dma_start(out=mf, in_=m_param.ap()[b * 128:(b + 1) * 128, :])
                nc.vector.tensor_copy(mb, mf)
                nc.sync.dma_start(out=mbf_dram[b * 128:(b + 1) * 128, :], in_=mb)

            # ---- M^T via xbar DMA transpose (bf16)
            for c in range(8):
                nc.sync.dma_start_transpose(
                    out=mt_sb[:, c, :],
                    in_=mbf_dram[:, c * 128:(c + 1) * 128],
                )

            # ---- one-hot H[u, j] = (his[j] == u), u = c*128+p
            for c in range(8):
                nc.vector.tensor_scalar(
                    out=h_sb[:, c, :],
                    in0=his_sb,
                    scalar1=ucol_sb[:, c:c + 1],
                    scalar2=None,
                    op0=mybir.AluOpType.is_equal,
                )

            # ---- G = M @ H on PE (bf16, f32 accum), exp on ScalarE
            for m in range(8):          # t block (psum partition = t%128)
                for n in range(2):      # j half (512 wide)
                    pg = psum.tile([128, 512], F32)
                    for c in range(8):  # contraction over u
                        nc.tensor.matmul(
                            pg,
                            mt_sb[:, c, m * 128:(m + 1) * 128],
                            h_sb[:, c, n * 512:(n + 1) * 512],
                            start=(c == 0),
                            stop=(c == 7),
                        )
                    nc.scalar.activation(
                        out=eg_sb[:, m, n * 512:(n + 1) * 512],
                        in_=pg,
                        func=mybir.ActivationFunctionType.Exp,
                    )
                # rowsum partial over this core's j shard
                nc.vector.reduce_sum(
                    out=rs_sb[:, m:m + 1],
                    in_=eg_sb[:, m, :],
                    axis=mybir.AxisListType.X,
                )

            # ---- AllReduce row-sum partials (4 KiB)
            nc.gpsimd.dma_start(out=cc_in[:], in_=rs_sb)
            nc.gpsimd.collective_compute(
                "AllReduce",
                mybir.AluOpType.add,
                replica_groups=[list(range(NCORES))],
                ins=[cc_in.opt()],
                outs=[cc_out.opt()],
            )
            nc.gpsimd.dma_start(out=rs_sb[:], in_=cc_out[:])
            nc.vector.reciprocal(out=inv_sb, in_=rs_sb)

            # ---- S = EG * (1/rowsum), park in DRAM as bf16
            for m in range(8):
                nc.vector.tensor_scalar_mul(
                    eg_sb[:, m, :], eg_sb[:, m, :], inv_sb[:, m:m + 1]
                )
            nc.sync.dma_start(
                out=s_dram[:].rearrange("(m p) j -> p m j", p=128),
                in_=eg_sb,
            )

            # ---- gather rows by cur, cast bf16->f32 while storing
            for ch in range(NCHUNKS):
                g = gat.tile([128, GCHUNK // 128, JSH], BF16)
                nc.gpsimd.dma_gather(
                    g,
                    s_dram[:],
                    idx_sb[:, ch * (GCHUNK // 16):(ch + 1) * (GCHUNK // 16)],
                    num_idxs=GCHUNK,
                    num_idxs_reg=GCHUNK,
                    elem_size=JSH,
                    elem_step=JSH,
                )
                out_view = out_param.ap()[ch * GCHUNK:(ch + 1) * GCHUNK, :]
                nc.gpsimd.dma_start(
                    out=out_view.rearrange("(q p) j -> p q j", p=128),
                    in_=g,
                )

    nc.compile()
    return nc


_NC_CACHE = None
_last_in_maps = None


def _get_nc():
    global _NC_CACHE
    if _NC_CACHE is None:
        _NC_CACHE = build_kernel()
    return _NC_CACHE


def kernel(his, cur, time_sim_mat):
    his = np.asarray(his)
    cur = np.asarray(cur)
    m = np.ascontiguousarray(np.asarray(time_sim_mat, dtype=np.float32))

    # cur indices, wrapped for dma_gather: chunk ch uses columns
    # [ch*32, (ch+1)*32); index g of a chunk sits at [g%16, g//16].
    a = np.zeros((16, STATE // 16), dtype=np.int16)
    for ch in range(NCHUNKS):
        blk = cur[ch * GCHUNK:(ch + 1) * GCHUNK].astype(np.int16)
        a[:, ch * (GCHUNK // 16):(ch + 1) * (GCHUNK // 16)] = (
            blk.reshape(GCHUNK // 16, 16).T
        )
    cur16 = np.tile(a, (8, 1))  # replicate across the 8 gpsimd core groups

    p = np.arange(128, dtype=np.float32)
    ucol = (p[:, None] + 128.0 * np.arange(8, dtype=np.float32)[None, :])
    ucol = np.ascontiguousarray(ucol)

    in_maps = []
    for k in range(NCORES):
        in_maps.append({
            "time_sim_mat": m,
            "his_f32": np.ascontiguousarray(
                his[k * JSH:(k + 1) * JSH].astype(np.float32)),
            "cur_idx16": cur16,
            "ucol": ucol,
        })

    global _last_in_maps
    _last_in_maps = in_maps

    nc = _get_nc()
    res = run_bass_kernel_spmd(nc, in_maps, core_ids=list(range(NCORES)))
    out = np.concatenate([res.results[k]["out"] for k in range(NCORES)], axis=1)
    return out


# revision 4
# speedup vs baseline: 1.7184x; 1.7184x over previous
"""Trainium2 Bass kernel for nn_Attn_time (sparse time-similarity attention).

reference:
    energies[i, j] = time_sim_mat[cur[i], his[j]]   # [4096, 8192]
    out = softmax(energies, axis=-1)

Structure exploited: cur/his index into only T=1024 time buckets, so
    out[i, j] = S[cur[i], j]  where  S = softmax_rows(time_sim_mat[:, his])
and S is only [1024, 8192]. Column-shard S across the 8 cores (1024 j each):

 - G[t, j] = sum_u M[t, u] * H[u, j] with one-hot H[u, j] = (his[j] == u),
   run on the TensorEngine in bf16. M is split host-side into two bf16
   planes (M = Mh + Ml, lossless to ~2^-17) and both planes' matmuls
   accumulate into the same PSUM tile, so energies are f32-accurate.
 - softmax denominator: rowsum[t] = sum_u exp(M[t, u]) * cnt[u] where
   cnt = bincount(his) (host-side index preprocessing). This makes every
   core's softmax fully local - no collectives.
 - S rows (bf16) are parked in DRAM and row-gathered by `cur` with
   dma_gather; the output store casts bf16->f32 in the SWDGE DMA.
Per-core output shard: out[:, k*1024:(k+1)*1024]; host concatenates.
"""

import numpy as np

import concourse.bass as bass
import concourse.tile as tile
from concourse import bacc, mybir
from concourse.bass_utils import run_bass_kernel_spmd

T = 1024          # time buckets
SEQ = 8192        # len(his)
STATE = 4096      # len(cur)
NCORES = 8
JSH = SEQ // NCORES        # j columns per core = 1024
GCHUNK = 1024              # gather rows per dma_gather chunk
NCHUNKS = STATE // GCHUNK  # 4

F32 = mybir.dt.float32
BF16 = mybir.dt.bfloat16
I16 = mybir.dt.int16


def build_kernel():
    nc = bacc.Bacc("TRN2", target_bir_lowering=False, debug=False,
                   num_devices=NCORES)

    mh_param = nc.dram_tensor("mh", [T, T], BF16, kind="ExternalInput")
    ml_param = nc.dram_tensor("ml", [T, T], BF16, kind="ExternalInput")
    his_param = nc.dram_tensor("his_f32", [JSH], F32, kind="ExternalInput")
    cur_param = nc.dram_tensor("cur_idx16", [128, STATE // 16], I16,
                               kind="ExternalInput")
    ucol_param = nc.dram_tensor("ucol", [128, 8], F32, kind="ExternalInput")
    cnt_param = nc.dram_tensor("cnt_bf16", [128, 8], BF16, kind="ExternalInput")
    out_param = nc.dram_tensor("out", [STATE, JSH], F32, kind="ExternalOutput")

    with tile.TileContext(nc, num_cores=NCORES) as tc:
        with (
            tc.tile_pool(name="singles", bufs=1) as singles,
            tc.tile_pool(name="gat", bufs=3) as gat,
            tc.tile_pool(name="psum", bufs=2, space="PSUM") as psum,
            tc.tile_pool(name="psrs", bufs=1, space="PSUM") as psrs,
            tc.tile_pool(name="dram", bufs=1, space="DRAM") as dram,
        ):
            # ---- persistent SBUF tiles
            mht_sb = singles.tile([128, 8, T], BF16)     # Mh^T [u%128, u//128, t]
            mlt_sb = singles.tile([128, 8, T], BF16)     # Ml^T
            mexp_sb = singles.tile([128, 8, T], BF16)    # exp(Mh^T)
            h_sb = singles.tile([128, 8, JSH], BF16)     # one-hot his shard
            eg_sb = singles.tile([128, 8, JSH], BF16)    # exp(G) -> S, t=m*128+p
            his_sb = singles.tile([128, JSH], F32)       # his bcast to all parts
            idx_sb = singles.tile([128, STATE // 16], I16)
            ucol_sb = singles.tile([128, 8], F32)        # ucol[p,c] = c*128+p
            cnt_sb = singles.tile([128, 8], BF16)        # bincount(his)
            rs_sb = singles.tile([128, 8], F32)          # rowsum
            inv_sb = singles.tile([128, 8], F32)         # 1/rowsum

            s_dram = dram.tile([T, JSH], BF16)

            # ---- small input loads
            nc.sync.dma_start(out=idx_sb, in_=cur_param.ap())
            nc.sync.dma_start(out=ucol_sb, in_=ucol_param.ap())
            nc.sync.dma_start(out=cnt_sb, in_=cnt_param.ap())
            nc.sync.dma_start(
                out=his_sb,
                in_=bass.AP(tensor=his_param, offset=0, ap=[[0, 128], [1, JSH]]),
            )

            # ---- M^T planes via xbar DMA transpose (bf16)
            for c in range(8):
                nc.sync.dma_start_transpose(
                    out=mht_sb[:, c, :],
                    in_=mh_param.ap()[:, c * 128:(c + 1) * 128],
                )
                nc.sync.dma_start_transpose(
                    out=mlt_sb[:, c, :],
                    in_=ml_param.ap()[:, c * 128:(c + 1) * 128],
                )

            # ---- one-hot H[u, j] = (his[j] == u), u = c*128+p
            for c in range(8):
                nc.vector.tensor_scalar(
                    out=h_sb[:, c, :],
                    in0=his_sb,
                    scalar1=ucol_sb[:, c:c + 1],
                    scalar2=None,
                    op0=mybir.AluOpType.is_equal,
                )

            # ---- local softmax denominator: rowsum = exp(Mh)^T' @ cnt
            for c in range(8):
                nc.scalar.activation(
                    out=mexp_sb[:, c, :],
                    in_=mht_sb[:, c, :],
                    func=mybir.ActivationFunctionType.Exp,
                )
            prs = psrs.tile([128, 8], F32)
            for m in range(8):
                for c in range(8):
                    nc.tensor.matmul(
                        prs[:, m:m + 1],
                        mexp_sb[:, c, m * 128:(m + 1) * 128],
                        cnt_sb[:, c:c + 1],
                        start=(c == 0),
                        stop=(c == 7),
                    )
            nc.vector.tensor_copy(rs_sb, prs)
            nc.vector.reciprocal(out=inv_sb, in_=rs_sb)

            # ---- G = (Mh + Ml) @ H on PE (bf16 x2, f32 accum), exp, scale
            for m in range(8):          # t block (psum partition = t%128)
                for n in range(2):      # j half (512 wide)
                    pg = psum.tile([128, 512], F32)
                    for c in range(8):  # contraction over u, high plane
                        nc.tensor.matmul(
                            pg,
                            mht_sb[:, c, m * 128:(m + 1) * 128],
                            h_sb[:, c, n * 512:(n + 1) * 512],
                            start=(c == 0),
                            stop=False,
                        )
                    for c in range(8):  # low plane
                        nc.tensor.matmul(
                            pg,
                            mlt_sb[:, c, m * 128:(m + 1) * 128],
                            h_sb[:, c, n * 512:(n + 1) * 512],
                            start=False,
                            stop=(c == 7),
                        )
                    nc.scalar.activation(
                        out=eg_sb[:, m, n * 512:(n + 1) * 512],
                        in_=pg,
                        func=mybir.ActivationFunctionType.Exp,
                    )
                # S rows for this t block: scale and park in DRAM (bf16)
                nc.vector.tensor_scalar_mul(
                    eg_sb[:, m, :], eg_sb[:, m, :], inv_sb[:, m:m + 1]
                )
                nc.sync.dma_start(
                    out=s_dram[m * 128:(m + 1) * 128, :],
                    in_=eg_sb[:, m, :],
                )

            # ---- gather rows by cur, cast bf16->f32 while storing
            for ch in range(NCHUNKS):
                g = gat.tile([128, GCHUNK // 128, JSH], BF16)
                nc.gpsimd.dma_gather(
                    g,
                    s_dram[:],
                    idx_sb[:, ch * (GCHUNK // 16):(ch + 1) * (GCHUNK // 16)],
                    num_idxs=GCHUNK,
                    num_idxs_reg=GCHUNK,
                    elem_size=JSH,
                    elem_step=JSH,
                )
                out_view = out_param.ap()[ch * GCHUNK:(ch + 1) * GCHUNK, :]
                nc.gpsimd.dma_start(
                    out=out_view.rearrange("(q p) j -> p q j", p=128),
                    in_=g,
                )

    nc.compile()
    return nc


_NC_CACHE = None
_last_in_maps = None


def _get_nc():
    global _NC_CACHE
    if _NC_CACHE is None:
        _NC_CACHE = build_kernel()
    return _NC_CACHE


def _bf16(x):
    import ml_dtypes
    return np.asarray(x, dtype=ml_dtypes.bfloat16)


def kernel(his, cur, time_sim_mat):
    import ml_dtypes

    his = np.asarray(his)
    cur = np.asarray(cur)
    m = np.asarray(time_sim_mat, dtype=np.float32)

    # split M into two bf16 planes: M ~= Mh + Ml (error ~2^-17)
    mh = m.astype(ml_dtypes.bfloat16)
    ml = (m - mh.astype(np.float32)).astype(ml_dtypes.bfloat16)
    mh = np.ascontiguousarray(mh)
    ml = np.ascontiguousarray(ml)

    # cur indices, wrapped for dma_gather: chunk ch uses idx columns
    # [ch*64, (ch+1)*64); index g of a chunk sits at [g%16, g//16].
    a = np.zeros((16, STATE // 16), dtype=np.int16)
    w = GCHUNK // 16
    for ch in range(NCHUNKS):
        blk = cur[ch * GCHUNK:(ch + 1) * GCHUNK].astype(np.int16)
        a[:, ch * w:(ch + 1) * w] = blk.reshape(w, 16).T
    cur16 = np.tile(a, (8, 1))  # replicate across the 8 gpsimd core groups

    p = np.arange(128, dtype=np.float32)
    ucol = np.ascontiguousarray(
        p[:, None] + 128.0 * np.arange(8, dtype=np.float32)[None, :])

    # histogram of his -> [128, 8] bf16, cnt[u=c*128+p] at [p, c]
    cnt = np.bincount(np.asarray(his, dtype=np.int64), minlength=T).astype(
        np.float32).reshape(8, 128).T
    cnt16 = np.ascontiguousarray(cnt.astype(ml_dtypes.bfloat16))

    in_maps = []
    for k in range(NCORES):
        in_maps.append({
            "mh": mh,
            "ml": ml,
            "his_f32": np.ascontiguousarray(
                his[k * JSH:(k + 1) * JSH].astype(np.float32)),
            "cur_idx16": cur16,
            "ucol": ucol,
            "cnt_bf16": cnt16,
        })

    global _last_in_maps
    _last_in_maps = in_maps

    nc = _get_nc()
    res = run_bass_kernel_spmd(nc, in_maps, core_ids=list(range(NCORES)))
    out = np.concatenate([res.results[k]["out"] for k in range(NCORES)], axis=1)
    return out
